# revision 9
# baseline (speedup 1.0000x reference)
"""Trainium2 Bass kernel for nn_MinEncoderOrder (vq_codebook).

Data-parallel over 8 NeuronCores: batch 32768 -> 4096/core. Weights and the
(512,8) codebook replicated. The three global BatchNorms are handled with
on-device AllReduce of (sum, sumsq) scalars; the BN affine is folded into the
next layer's PSUM->SBUF activation pass (per-partition scale/bias), so matmuls
never stall on stats.

Layout: activations feature-major [F, B_tile] (features on partitions, batch on
the free dim, 512-column batch tiles). Weights host-pre-transposed to [K, M]
and used as the stationary matmul operand.

mish(x) = x * (q-1)/(q+1) with q = (exp(x)+1)^2  [exact identity]:
2 ACT passes (exp, square) + 4 DVE passes, with BN sum/sumsq picked up for
free via scalar_tensor_tensor accum_out.

VQ: per-feature sorted codebook -> quant = c0 + sum_i delta_i * [out >= mid_i],
which reproduces the reference's argmin-with-max-on-ties semantics.
"""

import sys
import numpy as np

for _p in ("/opt/trn_rl_repo", "/root/.axon_site/_ro/trn_rl_repo"):
    if _p not in sys.path:
        sys.path.append(_p)

from contextlib import ExitStack

import concourse.bass as bass
import concourse.tile as tile
from concourse import bacc, mybir
from concourse.bass_utils import run_bass_kernel_spmd

F32 = mybir.dt.float32
AF = mybir.ActivationFunctionType
ALU = mybir.AluOpType

B_TOTAL = 32768
N_CORES = 8
B_CORE = B_TOTAL // N_CORES          # 4096
BT = 512                              # batch tile (one PSUM bank of fp32)
N_BT = B_CORE // BT                   # 8
EPS = 1e-5
FB = 512

SL_SHAPES = {
    "sl1": [(300, 24), (150, 150), (50, 50)],
    "sl2": [(600, 300), (300, 300), (100, 100)],
    "sl3": [(600, 600), (300, 300), (100, 100)],
}

# Output-row blocking per layer (PSUM partition chunks, aligned to the split
# boundary where that is free).
M_BLOCKS = {
    "sl1_0": [128, 22, 128, 22], "sl1_1": [100, 50], "sl1_2": [50],
    "sl2_0": [128, 128, 44, 128, 128, 44], "sl2_1": [128, 72, 100],
    "sl2_2": [100],
    "sl3_0": [128, 128, 44, 128, 128, 44], "sl3_1": [128, 72, 100],
    "sl3_2": [100],
    "fc": [128, 128, 128, 128],
}
# Contraction-dim chunking (must match the rhs tile segmentation; every
# segment must start at partition 0 of its tile — matmul moving-operand
# base-partition restriction).
K_SEGS = {
    "sl1_0": [24], "sl1_1": [128, 22], "sl1_2": [50],
    "sl2_0": [128, 128, 44], "sl2_1": [128, 128, 44], "sl2_2": [100],
    "sl3_0": [128, 128, 128, 128, 88], "sl3_1": [128, 128, 44], "sl3_2": [100],
    "fc": [128, 128, 128, 128, 88],
}
W_NAME = {"sl1_0": "sl1_w0t", "sl1_1": "sl1_w1t", "sl1_2": "sl1_w2t",
          "sl2_0": "sl2_w0t", "sl2_1": "sl2_w1t", "sl2_2": "sl2_w2t",
          "sl3_0": "sl3_w0t", "sl3_1": "sl3_w1t", "sl3_2": "sl3_w2t",
          "fc": "fc_wt"}


class Vec:
    """Feature-major vector stored as row-block tiles [p<=128, BT]."""

    def __init__(self, tiles, sizes):
        self.tiles = tiles
        self.sizes = sizes
        self.offsets = np.concatenate([[0], np.cumsum(sizes)]).astype(int)
        self.total = int(self.offsets[-1])

    def segments(self, lo, hi):
        for i, sz in enumerate(self.sizes):
            b0, b1 = self.offsets[i], self.offsets[i + 1]
            s0, s1 = max(lo, b0), min(hi, b1)
            if s0 < s1:
                yield self.tiles[i], int(s0 - b0), int(s1 - b0), int(s0)


def _pairs(src: Vec, src_lo, src_hi, dst: Vec, dst_lo):
    out = []
    for st, t0, t1, g0 in src.segments(src_lo, src_hi):
        cur_g, cur_t, n = g0, t0, t1 - t0
        while n > 0:
            d_global = dst_lo + (cur_g - src_lo)
            for dt_, u0, u1, _ in dst.segments(d_global, d_global + n):
                out.append((st, cur_t, cur_t + (u1 - u0), dt_, u0, u1))
                cur_t += u1 - u0
                cur_g += u1 - u0
                n -= u1 - u0
                break  # re-evaluate segments from the new position
    return out


def build_program(mm_dt=F32):
    nc = bacc.Bacc(None, target_bir_lowering=False, num_devices=N_CORES)

    dram_in = {}

    def din(name, shape):
        dram_in[name] = nc.declare_dram_parameter(name, list(shape), F32,
                                                  isOutput=False)
        return dram_in[name]

    xt_d = din("xt", (24, B_CORE))
    for ln, shapes in SL_SHAPES.items():
        for i, (o, k) in enumerate(shapes):
            din(f"{ln}_w{i}t", (k, o))
            din(f"{ln}_b{i}", (o, 1))
    din("fc_wt", (600, FB))
    din("fc_b", (FB, 1))
    for n, rows in (("sl2_0", 600), ("sl3_0", 600), ("fc", FB)):
        din(f"{n}_rs", (rows, 1))
    mids_d = din("mids", (FB, 7))
    c0_d = din("c0", (FB, 1))
    dts_d = din("dts", (FB, 7))

    out_d = nc.declare_dram_parameter("out_t", [FB, B_CORE], F32, isOutput=True)
    qnt_d = nc.declare_dram_parameter("qnt_t", [FB, B_CORE], F32, isOutput=True)

    core_ids = list(range(N_CORES))
    BIAS_NAME = {"sl1_0": "sl1_b0", "sl1_1": "sl1_b1", "sl1_2": "sl1_b2",
                 "sl2_0": "sl2_b0", "sl2_1": "sl2_b1", "sl2_2": "sl2_b2",
                 "sl3_0": "sl3_b0", "sl3_1": "sl3_b1", "sl3_2": "sl3_b2",
                 "fc": "fc_b"}

    with tile.TileContext(nc) as tc, ExitStack() as ctx:
        wpool = ctx.enter_context(tc.tile_pool(name="w", bufs=1))
        hpool = ctx.enter_context(tc.tile_pool(name="h", bufs=44))
        rpool = ctx.enter_context(tc.tile_pool(name="r", bufs=10))
        spool = ctx.enter_context(tc.tile_pool(name="s", bufs=2))
        opool = ctx.enter_context(tc.tile_pool(name="o", bufs=2))
        stpool = ctx.enter_context(tc.tile_pool(name="st", bufs=1))
        dpool = ctx.enter_context(tc.tile_pool(name="d", bufs=1, space="DRAM"))
        psum = ctx.enter_context(tc.tile_pool(name="ps", bufs=6, space="PSUM"))
        pstat = ctx.enter_context(tc.tile_pool(name="pst", bufs=2, space="PSUM"))

        # ---- weights: one tile per K-chunk, rows matching K_SEGS ----
        wtiles = {}
        for name, segs in K_SEGS.items():
            d = dram_in[W_NAME[name]]
            tiles, r = [], 0
            for kk in segs:
                t = wpool.tile([kk, d.shape[1]], mm_dt, tag=f"w_{name}_{r}")
                nc.sync.dma_start(out=t, in_=d[r:r + kk, :])
                tiles.append(t)
                r += kk
            assert r == d.shape[0]
            wtiles[name] = tiles

        # ---- per-M-chunk bias tiles ----
        btiles = {}
        for name, mbs in M_BLOCKS.items():
            d = dram_in[BIAS_NAME[name]]
            tiles, m0 = [], 0
            for mb in mbs:
                t = wpool.tile([mb, 1], F32, tag=f"b_{name}_{m0}")
                nc.sync.dma_start(out=t, in_=d[m0:m0 + mb, :])
                tiles.append(t)
                m0 += mb
            btiles[name] = tiles

        # ---- per-M-chunk rowsum tiles (BN-folded layers) ----
        rstiles = {}
        for name in ("sl2_0", "sl3_0", "fc"):
            d = dram_in[f"{name}_rs"]
            tiles, m0 = [], 0
            for mb in M_BLOCKS[name]:
                t = wpool.tile([mb, 1], F32, tag=f"rs_{name}_{m0}")
                nc.sync.dma_start(out=t, in_=d[m0:m0 + mb, :])
                tiles.append(t)
                m0 += mb
            rstiles[name] = tiles

        # ---- VQ constants, 128-row chunks (fc M blocks are 128-aligned) ----
        def load_chunks(d, width, tag):
            tiles = []
            for i in range(FB // 128):
                t = wpool.tile([128, width], F32, tag=f"{tag}_{i}")
                nc.sync.dma_start(out=t, in_=d[i * 128:(i + 1) * 128, :])
                tiles.append(t)
            return tiles

        mids_t = load_chunks(mids_d, 7, "mids")
        c0_t = load_chunks(c0_d, 1, "c0")
        dts_t = load_chunks(dts_d, 7, "dts")

        ones_t = wpool.tile([128, 1], F32, tag="ones")
        nc.vector.memset(ones_t, 1.0)

        # ---- stats staging ----
        NSTAT = 352
        stats_t = stpool.tile([128, NSTAT], F32, tag="stats")
        nc.vector.memset(stats_t, 0.0)
        stat_col = [0]
        stat_ranges = {}

        def stat_cols(skey):
            c = stat_col[0]
            stat_col[0] += 1
            assert stat_col[0] <= NSTAT
            lo, hi = stat_ranges.get(skey, (c, c))
            stat_ranges[skey] = (min(lo, c), c + 1)
            return c

        s_bcast = [wpool.tile([128, 1], F32, tag=f"s_b{k}", name=f"s_b{k}") for k in range(3)]
        sm_bcast = [wpool.tile([128, 1], F32, tag=f"sm_b{k}", name=f"sm_b{k}") for k in range(3)]

        # ---------------- helpers ----------------
        def matmul_layer(name, rhs_vec, rhs_lo, rhs_hi):
            segs = list(rhs_vec.segments(rhs_lo, rhs_hi))
            assert [t1 - t0 for _, t0, t1, _ in segs] == K_SEGS[name], \
                f"{name}: rhs segs {[t1 - t0 for _, t0, t1, _ in segs]}"
            ps_tiles, m0 = [], 0
            for mb in M_BLOCKS[name]:
                ps = psum.tile([128, BT], F32, tag="mm")
                for si, ((rt, t0, t1, _), wt) in enumerate(zip(segs, wtiles[name])):
                    nc.tensor.matmul(
                        ps[:mb, :], wt[:, m0:m0 + mb], rt[t0:t1, 0:BT],
                        start=(si == 0), stop=(si == len(segs) - 1))
                ps_tiles.append((ps, mb))
                m0 += mb
            return ps_tiles

        def relu_drain(ps_tiles, name, bn=None):
            tiles, sizes, m0 = [], [], 0
            for ci, (ps, mb) in enumerate(ps_tiles):
                rt = rpool.tile([128, BT], F32, tag="r")
                if bn is None:
                    nc.scalar.activation(rt[:mb, :], ps[:mb, :], AF.Relu,
                                         bias=btiles[name][ci][:, 0:1], scale=1.0)
                else:
                    k, beta = bn
                    nc.scalar.activation(rt[:mb, :], ps[:mb, :], AF.Relu,
                                         bias=beta[ci][:, 0:1],
                                         scale=s_bcast[k][:mb, 0:1])
                tiles.append(rt)
                sizes.append(mb)
                m0 += mb
            return Vec(tiles, sizes)

        def make_beta(name, k):
            """beta_chunk = bias - rowsum(W)*s*m per output feature."""
            betas = []
            for ci, mb in enumerate(M_BLOCKS[name]):
                bt_ = wpool.tile([mb, 1], F32, tag=f"beta_{name}_{ci}")
                nc.vector.scalar_tensor_tensor(
                    out=bt_[:, 0:1], in0=rstiles[name][ci][:, 0:1], scalar=-1.0,
                    in1=sm_bcast[k][:mb, 0:1], op0=ALU.mult, op1=ALU.mult)
                nc.vector.tensor_tensor(out=bt_[:, 0:1], in0=bt_[:, 0:1],
                                        in1=btiles[name][ci][:, 0:1], op=ALU.add)
                betas.append(bt_)
            return betas

        def mish_block(src, src_hi, dst, dst_lo, skey):
            """dst[dst_lo:dst_lo+src_hi) = mish(src[0:src_hi)).

            Compute ops only touch partition-0-based slices (hardware
            alignment rule); an SBUF->SBUF DMA repacks into the h layout."""
            for (st, t0, t1, g0) in src.segments(0, src_hi):
                assert t0 == 0
                p = t1 - t0
                xa = st[0:p, 0:BT]
                u = spool.tile([128, BT], F32, tag="msA")
                q = spool.tile([128, BT], F32, tag="msB")
                a = spool.tile([128, BT], F32, tag="msC")
                mo = spool.tile([128, BT], F32, tag="msD")
                nc.scalar.activation(u[:p, :], xa, AF.Exp)
                nc.scalar.activation(q[:p, :], u[:p, :], AF.Square, bias=1.0)
                nc.vector.tensor_scalar(out=a[:p, :], in0=q[:p, :], scalar1=1.0,
                                        scalar2=None, op0=ALU.add)
                nc.vector.reciprocal(out=a[:p, :], in_=a[:p, :])
                nc.vector.tensor_scalar(out=q[:p, :], in0=q[:p, :], scalar1=-1.0,
                                        scalar2=None, op0=ALU.add)
                nc.vector.scalar_tensor_tensor(out=q[:p, :], in0=q[:p, :],
                                               scalar=1.0, in1=a[:p, :],
                                               op0=ALU.mult, op1=ALU.mult)
                c_sum = stat_cols(skey)
                nc.vector.scalar_tensor_tensor(
                    out=mo[:p, :], in0=q[:p, :], scalar=1.0, in1=xa,
                    op0=ALU.mult, op1=ALU.mult,
                    accum_out=stats_t[:p, c_sum:c_sum + 1])
                c_sq = stat_cols(skey)
                nc.vector.scalar_tensor_tensor(
                    out=u[:p, :], in0=mo[:p, :], scalar=1.0, in1=mo[:p, :],
                    op0=ALU.mult, op1=ALU.mult,
                    accum_out=stats_t[:p, c_sq:c_sq + 1])
                d0 = dst_lo + g0
                for dt_, u0, u1, hg in dst.segments(d0, d0 + p):
                    nc.sync.dma_start(
                        out=dt_[u0:u1, 0:BT],
                        in_=mo[hg - d0:hg - d0 + (u1 - u0), 0:BT])

        def bn_reduce(k, skey, count):
            lo, hi = stat_ranges[skey]
            ncols = hi - lo
            nsum = pstat.tile([1, ncols], F32, tag="pst")
            nc.tensor.matmul(nsum[0:1, :], ones_t[:, 0:1], stats_t[:, lo:hi],
                             start=True, stop=True)
            srow = stpool.tile([1, ncols], F32, tag=f"srow{k}")
            nc.vector.tensor_copy(srow[0:1, :], nsum[0:1, :])
            tot = stpool.tile([1, 2], F32, tag=f"tot{k}")
            sview = srow[0:1, :].rearrange("p (n two) -> p n two", two=2)
            nc.vector.tensor_reduce(out=tot[0:1, 0:1], in_=sview[0:1, :, 0:1],
                                    axis=mybir.AxisListType.XY, op=ALU.add)
            nc.vector.tensor_reduce(out=tot[0:1, 1:2], in_=sview[0:1, :, 1:2],
                                    axis=mybir.AxisListType.XY, op=ALU.add)
            cc_in = dpool.tile([1, 2], F32, tag=f"ccin{k}")
            cc_out = dpool.tile([1, 2], F32, tag=f"ccout{k}")
            nc.sync.dma_start(out=cc_in[:, :], in_=tot[0:1, :])
            nc.gpsimd.collective_compute(
                "AllReduce", ALU.add, replica_groups=[core_ids],
                ins=[cc_in[:, :]], outs=[cc_out[:, :]])
            g = stpool.tile([1, 2], F32, tag=f"g{k}")
            nc.sync.dma_start(out=g[0:1, :], in_=cc_out[:, :])
            inv_n = 1.0 / float(count)
            mm_ = stpool.tile([1, 4], F32, tag=f"mm{k}")
            nc.vector.tensor_scalar(out=mm_[0:1, 0:2], in0=g[0:1, 0:2],
                                    scalar1=inv_n, scalar2=None, op0=ALU.mult)
            nc.vector.scalar_tensor_tensor(out=mm_[0:1, 2:3], in0=mm_[0:1, 0:1],
                                           scalar=-1.0, in1=mm_[0:1, 0:1],
                                           op0=ALU.mult, op1=ALU.mult)
            nc.vector.tensor_tensor(out=mm_[0:1, 2:3], in0=mm_[0:1, 2:3],
                                    in1=mm_[0:1, 1:2], op=ALU.add)
            nc.vector.tensor_scalar(out=mm_[0:1, 2:3], in0=mm_[0:1, 2:3],
                                    scalar1=EPS, scalar2=None, op0=ALU.add)
            # rsqrt(v): int bit-hack seed + 3 Newton iterations (all DVE)
            iv = mm_[0:1, 2:3]
            y = stpool.tile([1, 1], F32, tag=f"y{k}")
            nc.vector.tensor_scalar(out=y[0:1, 0:1].bitcast(mybir.dt.int32),
                                    in0=iv.bitcast(mybir.dt.int32), scalar1=1,
                                    scalar2=None, op0=ALU.logical_shift_right)
            nc.vector.tensor_scalar(out=y[0:1, 0:1].bitcast(mybir.dt.int32),
                                    in0=y[0:1, 0:1].bitcast(mybir.dt.int32),
                                    scalar1=-1, scalar2=None,
                                    op0=ALU.bitwise_xor)
            nc.vector.tensor_scalar(out=y[0:1, 0:1].bitcast(mybir.dt.int32),
                                    in0=y[0:1, 0:1].bitcast(mybir.dt.int32),
                                    scalar1=0x5F3759E0, scalar2=None,
                                    op0=ALU.add)
            t1_ = stpool.tile([1, 1], F32, tag=f"t1{k}")
            for _ in range(3):
                nc.vector.tensor_tensor(out=t1_[0:1, 0:1], in0=y[0:1, 0:1],
                                        in1=y[0:1, 0:1], op=ALU.mult)
                nc.vector.scalar_tensor_tensor(out=t1_[0:1, 0:1], in0=iv,
                                               scalar=-0.5, in1=t1_[0:1, 0:1],
                                               op0=ALU.mult, op1=ALU.mult)
                nc.vector.tensor_scalar(out=t1_[0:1, 0:1], in0=t1_[0:1, 0:1],
                                        scalar1=1.5, scalar2=None, op0=ALU.add)
                nc.vector.tensor_tensor(out=y[0:1, 0:1], in0=y[0:1, 0:1],
                                        in1=t1_[0:1, 0:1], op=ALU.mult)
            sm_ = stpool.tile([1, 2], F32, tag=f"smv{k}")
            nc.vector.tensor_copy(sm_[0:1, 0:1], y[0:1, 0:1])
            nc.vector.tensor_tensor(out=sm_[0:1, 1:2], in0=y[0:1, 0:1],
                                    in1=mm_[0:1, 0:1], op=ALU.mult)
            sc = dpool.tile([1, 2], F32, tag=f"sc{k}")
            nc.sync.dma_start(out=sc[:, :], in_=sm_[0:1, :])
            nc.sync.dma_start(out=s_bcast[k][:, 0:1],
                              in_=sc[0:1, 0:1].partition_broadcast(128))
            nc.sync.dma_start(out=sm_bcast[k][:, 0:1],
                              in_=sc[0:1, 1:2].partition_broadcast(128))

        # ---------------- the network ----------------
        def sl_phase(ln, in_vecs, h_sizes, bn_in, skey):
            """One split-layer block for all batch tiles -> dict of h Vecs."""
            h_all = {}
            o0, o1, o2 = {"sl1": (150, 100, 50), "sl2": (300, 200, 100),
                          "sl3": (300, 200, 100)}[ln]
            n0 = {"sl1": 300, "sl2": 600, "sl3": 600}[ln]
            n1 = {"sl1": 150, "sl2": 300, "sl3": 300}[ln]
            n2 = {"sl1": 50, "sl2": 100, "sl3": 100}[ln]
            for jb in range(N_BT):
                h = Vec([hpool.tile([128, BT], F32, tag="h", name="h") for _ in h_sizes],
                        h_sizes)
                h_all[jb] = h
                ps = matmul_layer(f"{ln}_0", in_vecs[jb], 0, in_vecs[jb].total)
                r0 = relu_drain(ps, f"{ln}_0", bn=bn_in)
                ps = matmul_layer(f"{ln}_1", r0, o0, n0)
                r1 = relu_drain(ps, f"{ln}_1")
                ps = matmul_layer(f"{ln}_2", r1, o1, n1)
                r2 = relu_drain(ps, f"{ln}_2")
                mish_block(r0, o0, h, 0, skey)
                mish_block(r1, o1, h, o0, skey)
                mish_block(r2, n2, h, o0 + o1, skey)
            return h_all

        # phase 1
        xvecs = {}
        for jb in range(N_BT):
            xt_t = rpool.tile([24, BT], F32, tag="xt", bufs=3)
            nc.sync.dma_start(out=xt_t, in_=xt_d[:, jb * BT:(jb + 1) * BT])
            xvecs[jb] = Vec([xt_t], [24])
        h1 = sl_phase("sl1", xvecs, [128, 128, 44], None, "bn1")
        bn_reduce(0, "bn1", B_TOTAL * 300)
        beta_sl2 = make_beta("sl2_0", 0)

        h2 = sl_phase("sl2", h1, [128, 128, 128, 128, 88], (0, beta_sl2), "bn2")
        bn_reduce(1, "bn2", B_TOTAL * 600)
        beta_sl3 = make_beta("sl3_0", 1)

        h3 = sl_phase("sl3", h2, [128, 128, 128, 128, 88], (1, beta_sl3), "bn3")
        bn_reduce(2, "bn3", B_TOTAL * 600)
        beta_fc = make_beta("fc", 2)

        # phase 4: fc + sigmoid + VQ
        for jb in range(N_BT):
            ps = matmul_layer("fc", h3[jb], 0, 600)
            m0 = 0
            for ci, (pst_, mb) in enumerate(ps):
                ot = opool.tile([128, BT], F32, tag="out")
                nc.scalar.activation(ot[:mb, :], pst_[:mb, :], AF.Sigmoid,
                                     bias=beta_fc[ci][:, 0:1],
                                     scale=s_bcast[2][:mb, 0:1])
                qa = opool.tile([128, BT], F32, tag="qA")
                qb = opool.tile([128, BT], F32, tag="qB")
                cmp = opool.tile([128, BT], F32, tag="qC")
                nc.vector.tensor_scalar(out=qa[:mb, :], in0=ot[:mb, :],
                                        scalar1=0.0, scalar2=c0_t[ci][:mb, 0:1],
                                        op0=ALU.mult, op1=ALU.add)
                src, dst = qa, qb
                for t in range(7):
                    nc.vector.tensor_scalar(out=cmp[:mb, :], in0=ot[:mb, :],
                                            scalar1=mids_t[ci][:mb, t:t + 1],
                                            scalar2=None, op0=ALU.is_ge)
                    nc.vector.scalar_tensor_tensor(
                        out=dst[:mb, :], in0=cmp[:mb, :],
                        scalar=dts_t[ci][:mb, t:t + 1], in1=src[:mb, :],
                        op0=ALU.mult, op1=ALU.add)
                    src, dst = dst, src
                nc.sync.dma_start(out=out_d[m0:m0 + mb, jb * BT:(jb + 1) * BT],
                                  in_=ot[:mb, :])
                nc.sync.dma_start(out=qnt_d[m0:m0 + mb, jb * BT:(jb + 1) * BT],
                                  in_=src[:mb, :])
                m0 += mb

    nc.finalize()
    return nc


_CACHE = {}


def _get_program():
    if "nc" not in _CACHE:
        _CACHE["nc"] = build_program()
    return _CACHE["nc"]


def _prep_inputs(inputs):
    f32 = np.float32
    x = np.ascontiguousarray(np.asarray(inputs["x"], f32))

    shared = {}
    for ln, shapes in SL_SHAPES.items():
        for i in range(len(shapes)):
            w = np.asarray(inputs[f"{ln}_w{i}"], f32)
            b = np.asarray(inputs[f"{ln}_b{i}"], f32)
            shared[f"{ln}_w{i}t"] = np.ascontiguousarray(w.T)
            shared[f"{ln}_b{i}"] = np.ascontiguousarray(b.reshape(-1, 1))
    fw = np.asarray(inputs["fc_w"], f32)
    shared["fc_wt"] = np.ascontiguousarray(fw.T)
    shared["fc_b"] = np.ascontiguousarray(
        np.asarray(inputs["fc_b"], f32).reshape(-1, 1))
    shared["sl2_0_rs"] = np.ascontiguousarray(
        np.asarray(inputs["sl2_w0"], f32).sum(axis=1).reshape(-1, 1))
    shared["sl3_0_rs"] = np.ascontiguousarray(
        np.asarray(inputs["sl3_w0"], f32).sum(axis=1).reshape(-1, 1))
    shared["fc_rs"] = np.ascontiguousarray(fw.sum(axis=1).reshape(-1, 1))
    cb = np.asarray(inputs["codebook"], f32)[0, :, 0, :]
    cs = np.sort(cb, axis=1)
    shared["mids"] = np.ascontiguousarray((cs[:, :-1] + cs[:, 1:]) * 0.5)
    shared["c0"] = np.ascontiguousarray(cs[:, 0:1])
    shared["dts"] = np.ascontiguousarray(cs[:, 1:] - cs[:, :-1])

    in_maps = []
    for c in range(N_CORES):
        m = dict(shared)
        m["xt"] = np.ascontiguousarray(x[c * B_CORE:(c + 1) * B_CORE, :].T)
        in_maps.append(m)
    return in_maps


def run(inputs, trace=False):
    nc = _get_program()
    in_maps = _prep_inputs(inputs)
    res = run_bass_kernel_spmd(nc, in_maps, list(range(N_CORES)), trace=trace)
    outs = np.empty((FB, B_TOTAL), np.float32)
    qnts = np.empty((FB, B_TOTAL), np.float32)
    for c in range(N_CORES):
        outs[:, c * B_CORE:(c + 1) * B_CORE] = res.results[c]["out_t"]
        qnts[:, c * B_CORE:(c + 1) * B_CORE] = res.results[c]["qnt_t"]
    out = np.ascontiguousarray(outs.T)
    qnt = np.ascontiguousarray(qnts.T)
    return (qnt.copy(), qnt.reshape(B_TOTAL, FB, 1),
            out.reshape(B_TOTAL, FB, 1)), res


def kernel(**inputs):
    (strite, qnt, out), _ = run(inputs, trace=False)
    return strite, qnt, out


# revision 19
# speedup vs baseline: 1.1761x; 1.1761x over previous
"""Trainium2 Bass kernel for nn_MinEncoderOrder (vq_codebook).

Data-parallel over 8 NeuronCores: batch 32768 -> 4096/core. Weights and the
(512,8) codebook replicated. The three global BatchNorms are handled with
on-device AllReduce of (sum, sumsq) scalars; the BN affine is folded into the
next layer's PSUM->SBUF activation pass (per-partition scale/bias), so matmuls
never stall on stats.

Layout: activations feature-major [F, B_tile] (features on partitions, batch on
the free dim, 512-column batch tiles). Weights host-pre-transposed to [K, M]
and used as the stationary matmul operand.

mish(x) = x * (q-1)/(q+1) with q = (exp(x)+1)^2  [exact identity]:
2 ACT passes (exp, square) + 4 DVE passes, with BN sum/sumsq picked up for
free via scalar_tensor_tensor accum_out.

VQ: per-feature sorted codebook -> quant = c0 + sum_i delta_i * [out >= mid_i],
which reproduces the reference's argmin-with-max-on-ties semantics.
"""

import sys
import numpy as np

for _p in ("/opt/trn_rl_repo", "/root/.axon_site/_ro/trn_rl_repo"):
    if _p not in sys.path:
        sys.path.append(_p)

from contextlib import ExitStack

import concourse.bass as bass
import concourse.tile as tile
from concourse import bacc, mybir
from concourse.bass_utils import run_bass_kernel_spmd

F32 = mybir.dt.float32
AF = mybir.ActivationFunctionType
ALU = mybir.AluOpType

B_TOTAL = 32768
N_CORES = 8
B_CORE = B_TOTAL // N_CORES          # 4096
BT = 512                              # batch tile (one PSUM bank of fp32)
N_BT = B_CORE // BT                   # 8
EPS = 1e-5
FB = 512

SL_SHAPES = {
    "sl1": [(300, 24), (150, 150), (50, 50)],
    "sl2": [(600, 300), (300, 300), (100, 100)],
    "sl3": [(600, 600), (300, 300), (100, 100)],
}

# Output-row blocking per layer (PSUM partition chunks, aligned to the split
# boundary where that is free).
M_BLOCKS = {
    "sl1_0": [128, 22, 128, 22], "sl1_1": [100, 50], "sl1_2": [50],
    "sl2_0": [128, 128, 44, 128, 128, 44], "sl2_1": [128, 72, 100],
    "sl2_2": [100],
    "sl3_0": [128, 128, 44, 128, 128, 44], "sl3_1": [128, 72, 100],
    "sl3_2": [100],
    "fc": [128, 128, 128, 128],
}
# Contraction-dim chunking (must match the rhs tile segmentation; every
# segment must start at partition 0 of its tile — matmul moving-operand
# base-partition restriction).
K_SEGS = {
    "sl1_0": [24], "sl1_1": [128, 22], "sl1_2": [50],
    "sl2_0": [128, 128, 44], "sl2_1": [128, 128, 44], "sl2_2": [100],
    "sl3_0": [128, 128, 128, 128, 88], "sl3_1": [128, 128, 44], "sl3_2": [100],
    "fc": [128, 128, 128, 128, 88],
}
W_NAME = {"sl1_0": "sl1_w0t", "sl1_1": "sl1_w1t", "sl1_2": "sl1_w2t",
          "sl2_0": "sl2_w0t", "sl2_1": "sl2_w1t", "sl2_2": "sl2_w2t",
          "sl3_0": "sl3_w0t", "sl3_1": "sl3_w1t", "sl3_2": "sl3_w2t",
          "fc": "fc_wt"}


class Vec:
    """Feature-major vector stored as row-block tiles [p<=128, BT]."""

    def __init__(self, tiles, sizes):
        self.tiles = tiles
        self.sizes = sizes
        self.offsets = np.concatenate([[0], np.cumsum(sizes)]).astype(int)
        self.total = int(self.offsets[-1])

    def segments(self, lo, hi):
        for i, sz in enumerate(self.sizes):
            b0, b1 = self.offsets[i], self.offsets[i + 1]
            s0, s1 = max(lo, b0), min(hi, b1)
            if s0 < s1:
                yield self.tiles[i], int(s0 - b0), int(s1 - b0), int(s0)


def _pairs(src: Vec, src_lo, src_hi, dst: Vec, dst_lo):
    out = []
    for st, t0, t1, g0 in src.segments(src_lo, src_hi):
        cur_g, cur_t, n = g0, t0, t1 - t0
        while n > 0:
            d_global = dst_lo + (cur_g - src_lo)
            for dt_, u0, u1, _ in dst.segments(d_global, d_global + n):
                out.append((st, cur_t, cur_t + (u1 - u0), dt_, u0, u1))
                cur_t += u1 - u0
                cur_g += u1 - u0
                n -= u1 - u0
                break  # re-evaluate segments from the new position
    return out


def build_program(f32r=True):
    """f32r=True streams matmul operands in float32r (12-bit mantissa,
    1 column/cycle) instead of fp32 (4 cycles/column)."""
    mm_dt = mybir.dt.float32r if f32r else F32
    nc = bacc.Bacc(None, target_bir_lowering=False, num_devices=N_CORES)

    dram_in = {}

    def din(name, shape, dt=F32):
        dram_in[name] = nc.declare_dram_parameter(name, list(shape), dt,
                                                  isOutput=False)
        return dram_in[name]

    xt_d = din("xt", (24, B_CORE), mm_dt)
    for ln, shapes in SL_SHAPES.items():
        for i, (o, k) in enumerate(shapes):
            din(f"{ln}_w{i}t", (k, o), mm_dt)
            din(f"{ln}_b{i}", (o, 1))
    din("fc_wt", (600, FB), mm_dt)
    din("fc_b", (FB, 1))
    for n, rows in (("sl2_0", 600), ("sl3_0", 600), ("fc", FB)):
        din(f"{n}_rs", (rows, 1))
    mids_d = din("mids", (FB, 7))
    c0_d = din("c0", (FB, 1))
    dts_d = din("dts", (FB, 7))

    out_d = nc.declare_dram_parameter("out_t", [FB, B_CORE], F32, isOutput=True)
    qnt_d = nc.declare_dram_parameter("qnt_t", [FB, B_CORE], F32, isOutput=True)

    core_ids = list(range(N_CORES))
    BIAS_NAME = {"sl1_0": "sl1_b0", "sl1_1": "sl1_b1", "sl1_2": "sl1_b2",
                 "sl2_0": "sl2_b0", "sl2_1": "sl2_b1", "sl2_2": "sl2_b2",
                 "sl3_0": "sl3_b0", "sl3_1": "sl3_b1", "sl3_2": "sl3_b2",
                 "fc": "fc_b"}

    with tile.TileContext(nc) as tc, ExitStack() as ctx:
        wpool = ctx.enter_context(tc.tile_pool(name="w", bufs=1))
        hpool = ctx.enter_context(tc.tile_pool(name="h", bufs=44))
        rpool = ctx.enter_context(tc.tile_pool(name="r", bufs=10))
        spool = ctx.enter_context(tc.tile_pool(name="s", bufs=2))
        opool = ctx.enter_context(tc.tile_pool(name="o", bufs=2))
        stpool = ctx.enter_context(tc.tile_pool(name="st", bufs=1))
        dpool = ctx.enter_context(tc.tile_pool(name="d", bufs=1, space="DRAM"))
        psum = ctx.enter_context(tc.tile_pool(name="ps", bufs=6, space="PSUM"))
        pstat = ctx.enter_context(tc.tile_pool(name="pst", bufs=2, space="PSUM"))

        # ---- weights: one tile per K-chunk, rows matching K_SEGS ----
        wtiles = {}
        for name, segs in K_SEGS.items():
            d = dram_in[W_NAME[name]]
            tiles, r = [], 0
            for kk in segs:
                t = wpool.tile([kk, d.shape[1]], mm_dt, tag=f"w_{name}_{r}")
                nc.sync.dma_start(out=t, in_=d[r:r + kk, :])
                tiles.append(t)
                r += kk
            assert r == d.shape[0]
            wtiles[name] = tiles

        # ---- per-M-chunk bias tiles ----
        btiles = {}
        for name, mbs in M_BLOCKS.items():
            d = dram_in[BIAS_NAME[name]]
            tiles, m0 = [], 0
            for mb in mbs:
                t = wpool.tile([mb, 1], F32, tag=f"b_{name}_{m0}")
                nc.sync.dma_start(out=t, in_=d[m0:m0 + mb, :])
                tiles.append(t)
                m0 += mb
            btiles[name] = tiles

        # ---- per-M-chunk rowsum tiles (BN-folded layers) ----
        rstiles = {}
        for name in ("sl2_0", "sl3_0", "fc"):
            d = dram_in[f"{name}_rs"]
            tiles, m0 = [], 0
            for mb in M_BLOCKS[name]:
                t = wpool.tile([mb, 1], F32, tag=f"rs_{name}_{m0}")
                nc.sync.dma_start(out=t, in_=d[m0:m0 + mb, :])
                tiles.append(t)
                m0 += mb
            rstiles[name] = tiles

        # ---- VQ constants, 128-row chunks (fc M blocks are 128-aligned) ----
        def load_chunks(d, width, tag):
            tiles = []
            for i in range(FB // 128):
                t = wpool.tile([128, width], F32, tag=f"{tag}_{i}")
                nc.sync.dma_start(out=t, in_=d[i * 128:(i + 1) * 128, :])
                tiles.append(t)
            return tiles

        mids_t = load_chunks(mids_d, 7, "mids")
        c0_t = load_chunks(c0_d, 1, "c0")
        dts_t = load_chunks(dts_d, 7, "dts")

        ones_t = wpool.tile([128, 1], F32, tag="ones")
        nc.vector.memset(ones_t, 1.0)

        # ---- stats staging ----
        NSTAT = 352
        stats_t = stpool.tile([128, NSTAT], F32, tag="stats")
        nc.vector.memset(stats_t, 0.0)
        stat_col = [0]
        stat_ranges = {}

        def stat_cols(skey):
            c = stat_col[0]
            stat_col[0] += 1
            assert stat_col[0] <= NSTAT
            lo, hi = stat_ranges.get(skey, (c, c))
            stat_ranges[skey] = (min(lo, c), c + 1)
            return c

        s_bcast = [wpool.tile([128, 1], F32, tag=f"s_b{k}", name=f"s_b{k}") for k in range(3)]
        sm_bcast = [wpool.tile([128, 1], F32, tag=f"sm_b{k}", name=f"sm_b{k}") for k in range(3)]

        # ---------------- helpers ----------------
        def matmul_layer(name, rhs_vec, rhs_lo, rhs_hi):
            segs = list(rhs_vec.segments(rhs_lo, rhs_hi))
            assert [t1 - t0 for _, t0, t1, _ in segs] == K_SEGS[name], \
                f"{name}: rhs segs {[t1 - t0 for _, t0, t1, _ in segs]}"
            ps_tiles, m0 = [], 0
            for mb in M_BLOCKS[name]:
                ps = psum.tile([128, BT], F32, tag="mm")
                for si, ((rt, t0, t1, _), wt) in enumerate(zip(segs, wtiles[name])):
                    nc.tensor.matmul(
                        ps[:mb, :], wt[:, m0:m0 + mb], rt[t0:t1, 0:BT],
                        start=(si == 0), stop=(si == len(segs) - 1))
                ps_tiles.append((ps, mb))
                m0 += mb
            return ps_tiles

        def relu_drain(ps_tiles, name, bn=None):
            tiles, sizes, m0 = [], [], 0
            for ci, (ps, mb) in enumerate(ps_tiles):
                rt = rpool.tile([128, BT], mm_dt, tag="r", name="r")
                if bn is None:
                    nc.scalar.activation(rt[:mb, :], ps[:mb, :], AF.Relu,
                                         bias=btiles[name][ci][:, 0:1], scale=1.0)
                else:
                    k, beta = bn
                    nc.scalar.activation(rt[:mb, :], ps[:mb, :], AF.Relu,
                                         bias=beta[ci][:, 0:1],
                                         scale=s_bcast[k][:mb, 0:1])
                tiles.append(rt)
                sizes.append(mb)
                m0 += mb
            return Vec(tiles, sizes)

        def make_beta(name, k):
            """beta_chunk = bias - rowsum(W)*s*m per output feature."""
            betas = []
            for ci, mb in enumerate(M_BLOCKS[name]):
                bt_ = wpool.tile([mb, 1], F32, tag=f"beta_{name}_{ci}")
                nc.vector.scalar_tensor_tensor(
                    out=bt_[:, 0:1], in0=rstiles[name][ci][:, 0:1], scalar=-1.0,
                    in1=sm_bcast[k][:mb, 0:1], op0=ALU.mult, op1=ALU.mult)
                nc.vector.tensor_tensor(out=bt_[:, 0:1], in0=bt_[:, 0:1],
                                        in1=btiles[name][ci][:, 0:1], op=ALU.add)
                betas.append(bt_)
            return betas

        def mish_block(src, src_hi, dst, dst_lo, skey):
            """dst[dst_lo:dst_lo+src_hi) = -mish(src[0:src_hi)).

            -mish(x) = x*(1 - 2/(1+sigmoid(-x)^2)) for x>=0; the sign is
            compensated by host-side negation of the consumer weights.
            Compute ops only touch partition-0-based slices (hardware
            alignment rule); an SBUF->SBUF DMA repacks into the h layout."""
            for (st, t0, t1, g0) in src.segments(0, src_hi):
                assert t0 == 0
                p = t1 - t0
                xa = st[0:p, 0:BT]
                if f32r:
                    # non-matmul engines read the (already-rounded) bits as f32
                    xa = xa.bitcast(F32)
                u = spool.tile([128, BT], F32, tag="msA", name="msA")
                q = spool.tile([128, BT], F32, tag="msB", name="msB")
                a = spool.tile([128, BT], F32, tag="msC", name="msC")
                mo = spool.tile([128, BT], mm_dt, tag="msD", name="msD")
                nc.scalar.activation(u[:p, :], xa, AF.Sigmoid, scale=-1.0)
                nc.scalar.activation(q[:p, :], u[:p, :], AF.Square)
                nc.vector.tensor_scalar(out=a[:p, :], in0=q[:p, :], scalar1=1.0,
                                        scalar2=None, op0=ALU.add)
                nc.vector.reciprocal(out=a[:p, :], in_=a[:p, :])
                nc.vector.tensor_scalar(out=a[:p, :], in0=a[:p, :], scalar1=-2.0,
                                        scalar2=1.0, op0=ALU.mult, op1=ALU.add)
                c_sum = stat_cols(skey)
                nc.vector.scalar_tensor_tensor(
                    out=mo[:p, :], in0=a[:p, :], scalar=1.0, in1=xa,
                    op0=ALU.mult, op1=ALU.mult,
                    accum_out=stats_t[:p, c_sum:c_sum + 1])
                c_sq = stat_cols(skey)
                mo_r = mo[:p, :].bitcast(F32) if f32r else mo[:p, :]
                nc.vector.scalar_tensor_tensor(
                    out=u[:p, :], in0=mo_r, scalar=1.0, in1=mo_r,
                    op0=ALU.mult, op1=ALU.mult,
                    accum_out=stats_t[:p, c_sq:c_sq + 1])
                d0 = dst_lo + g0
                for dt_, u0, u1, hg in dst.segments(d0, d0 + p):
                    nc.sync.dma_start(
                        out=dt_[u0:u1, 0:BT],
                        in_=mo[hg - d0:hg - d0 + (u1 - u0), 0:BT])

        def bn_reduce(k, skey, count):
            lo, hi = stat_ranges[skey]
            ncols = hi - lo
            nsum = pstat.tile([1, ncols], F32, tag="pst")
            nc.tensor.matmul(nsum[0:1, :], ones_t[:, 0:1], stats_t[:, lo:hi],
                             start=True, stop=True)
            srow = stpool.tile([1, ncols], F32, tag=f"srow{k}")
            nc.vector.tensor_copy(srow[0:1, :], nsum[0:1, :])
            tot = stpool.tile([1, 2], F32, tag=f"tot{k}")
            sview = srow[0:1, :].rearrange("p (n two) -> p n two", two=2)
            nc.vector.tensor_reduce(out=tot[0:1, 0:1], in_=sview[0:1, :, 0:1],
                                    axis=mybir.AxisListType.XY, op=ALU.add)
            nc.vector.tensor_reduce(out=tot[0:1, 1:2], in_=sview[0:1, :, 1:2],
                                    axis=mybir.AxisListType.XY, op=ALU.add)
            cc_in = dpool.tile([1, 2], F32, tag=f"ccin{k}")
            cc_out = dpool.tile([1, 2], F32, tag=f"ccout{k}")
            nc.sync.dma_start(out=cc_in[:, :], in_=tot[0:1, :])
            nc.gpsimd.collective_compute(
                "AllReduce", ALU.add, replica_groups=[core_ids],
                ins=[cc_in[:, :]], outs=[cc_out[:, :]])
            g = stpool.tile([1, 2], F32, tag=f"g{k}")
            nc.sync.dma_start(out=g[0:1, :], in_=cc_out[:, :])
            inv_n = 1.0 / float(count)
            mm_ = stpool.tile([1, 4], F32, tag=f"mm{k}")
            nc.vector.tensor_scalar(out=mm_[0:1, 0:2], in0=g[0:1, 0:2],
                                    scalar1=inv_n, scalar2=None, op0=ALU.mult)
            nc.vector.scalar_tensor_tensor(out=mm_[0:1, 2:3], in0=mm_[0:1, 0:1],
                                           scalar=-1.0, in1=mm_[0:1, 0:1],
                                           op0=ALU.mult, op1=ALU.mult)
            nc.vector.tensor_tensor(out=mm_[0:1, 2:3], in0=mm_[0:1, 2:3],
                                    in1=mm_[0:1, 1:2], op=ALU.add)
            nc.vector.tensor_scalar(out=mm_[0:1, 2:3], in0=mm_[0:1, 2:3],
                                    scalar1=EPS, scalar2=None, op0=ALU.add)
            # rsqrt(v): int bit-hack seed + 3 Newton iterations (all DVE)
            iv = mm_[0:1, 2:3]
            y = stpool.tile([1, 1], F32, tag=f"y{k}")
            nc.vector.tensor_scalar(out=y[0:1, 0:1].bitcast(mybir.dt.int32),
                                    in0=iv.bitcast(mybir.dt.int32), scalar1=1,
                                    scalar2=None, op0=ALU.logical_shift_right)
            nc.vector.tensor_scalar(out=y[0:1, 0:1].bitcast(mybir.dt.int32),
                                    in0=y[0:1, 0:1].bitcast(mybir.dt.int32),
                                    scalar1=-1, scalar2=None,
                                    op0=ALU.bitwise_xor)
            nc.vector.tensor_scalar(out=y[0:1, 0:1].bitcast(mybir.dt.int32),
                                    in0=y[0:1, 0:1].bitcast(mybir.dt.int32),
                                    scalar1=0x5F3759E0, scalar2=None,
                                    op0=ALU.add)
            t1_ = stpool.tile([1, 1], F32, tag=f"t1{k}")
            for _ in range(3):
                nc.vector.tensor_tensor(out=t1_[0:1, 0:1], in0=y[0:1, 0:1],
                                        in1=y[0:1, 0:1], op=ALU.mult)
                nc.vector.scalar_tensor_tensor(out=t1_[0:1, 0:1], in0=iv,
                                               scalar=-0.5, in1=t1_[0:1, 0:1],
                                               op0=ALU.mult, op1=ALU.mult)
                nc.vector.tensor_scalar(out=t1_[0:1, 0:1], in0=t1_[0:1, 0:1],
                                        scalar1=1.5, scalar2=None, op0=ALU.add)
                nc.vector.tensor_tensor(out=y[0:1, 0:1], in0=y[0:1, 0:1],
                                        in1=t1_[0:1, 0:1], op=ALU.mult)
            sm_ = stpool.tile([1, 2], F32, tag=f"smv{k}")
            nc.vector.tensor_copy(sm_[0:1, 0:1], y[0:1, 0:1])
            nc.vector.tensor_tensor(out=sm_[0:1, 1:2], in0=y[0:1, 0:1],
                                    in1=mm_[0:1, 0:1], op=ALU.mult)
            sc = dpool.tile([1, 2], F32, tag=f"sc{k}")
            nc.sync.dma_start(out=sc[:, :], in_=sm_[0:1, :])
            nc.sync.dma_start(out=s_bcast[k][:, 0:1],
                              in_=sc[0:1, 0:1].partition_broadcast(128))
            nc.sync.dma_start(out=sm_bcast[k][:, 0:1],
                              in_=sc[0:1, 1:2].partition_broadcast(128))

        # ---------------- the network ----------------
        def sl_phase(ln, in_vecs, h_sizes, bn_in, skey):
            """One split-layer block for all batch tiles -> dict of h Vecs."""
            h_all = {}
            o0, o1, o2 = {"sl1": (150, 100, 50), "sl2": (300, 200, 100),
                          "sl3": (300, 200, 100)}[ln]
            n0 = {"sl1": 300, "sl2": 600, "sl3": 600}[ln]
            n1 = {"sl1": 150, "sl2": 300, "sl3": 300}[ln]
            n2 = {"sl1": 50, "sl2": 100, "sl3": 100}[ln]
            for jb in range(N_BT):
                h = Vec([hpool.tile([128, BT], mm_dt, tag="h", name="h") for _ in h_sizes],
                        h_sizes)
                h_all[jb] = h
                ps = matmul_layer(f"{ln}_0", in_vecs[jb], 0, in_vecs[jb].total)
                r0 = relu_drain(ps, f"{ln}_0", bn=bn_in)
                ps = matmul_layer(f"{ln}_1", r0, o0, n0)
                r1 = relu_drain(ps, f"{ln}_1")
                ps = matmul_layer(f"{ln}_2", r1, o1, n1)
                r2 = relu_drain(ps, f"{ln}_2")
                mish_block(r0, o0, h, 0, skey)
                mish_block(r1, o1, h, o0, skey)
                mish_block(r2, n2, h, o0 + o1, skey)
            return h_all

        # phase 1
        xvecs = {}
        for jb in range(N_BT):
            xt_t = rpool.tile([24, BT], mm_dt, tag="xt", bufs=3)
            nc.sync.dma_start(out=xt_t, in_=xt_d[:, jb * BT:(jb + 1) * BT])
            xvecs[jb] = Vec([xt_t], [24])
        h1 = sl_phase("sl1", xvecs, [128, 128, 44], None, "bn1")
        bn_reduce(0, "bn1", B_TOTAL * 300)
        beta_sl2 = make_beta("sl2_0", 0)

        h2 = sl_phase("sl2", h1, [128, 128, 128, 128, 88], (0, beta_sl2), "bn2")
        bn_reduce(1, "bn2", B_TOTAL * 600)
        beta_sl3 = make_beta("sl3_0", 1)

        h3 = sl_phase("sl3", h2, [128, 128, 128, 128, 88], (1, beta_sl3), "bn3")
        bn_reduce(2, "bn3", B_TOTAL * 600)
        beta_fc = make_beta("fc", 2)

        # phase 4: fc + sigmoid + VQ
        for jb in range(N_BT):
            ps = matmul_layer("fc", h3[jb], 0, 600)
            m0 = 0
            for ci, (pst_, mb) in enumerate(ps):
                ot = opool.tile([128, BT], F32, tag="out")
                nc.scalar.activation(ot[:mb, :], pst_[:mb, :], AF.Sigmoid,
                                     bias=beta_fc[ci][:, 0:1],
                                     scale=s_bcast[2][:mb, 0:1])
                # quant = c0 + sum_i dts_i*[out >= mids_i]; fused compare+scale
                # at 2x TS rate, tree-sum split between DVE and GpSimd.
                qa = opool.tile([128, BT], F32, tag="qA", name="qA")
                qb = opool.tile([128, BT], F32, tag="qB", name="qB")
                qc = opool.tile([128, BT], F32, tag="qC", name="qC")

                def term(dst_t, t):
                    nc.vector.tensor_scalar(
                        out=dst_t[:mb, :], in0=ot[:mb, :],
                        scalar1=mids_t[ci][:mb, t:t + 1],
                        scalar2=dts_t[ci][:mb, t:t + 1],
                        op0=ALU.is_ge, op1=ALU.mult)

                term(qa, 0)
                term(qb, 1)
                nc.vector.tensor_tensor(out=qa[:mb, :], in0=qa[:mb, :],
                                        in1=qb[:mb, :], op=ALU.add)
                term(qb, 2)
                term(qc, 3)
                nc.vector.tensor_tensor(out=qb[:mb, :], in0=qb[:mb, :],
                                        in1=qc[:mb, :], op=ALU.add)
                nc.vector.tensor_tensor(out=qa[:mb, :], in0=qa[:mb, :],
                                        in1=qb[:mb, :], op=ALU.add)
                term(qb, 4)
                term(qc, 5)
                nc.vector.tensor_tensor(out=qb[:mb, :], in0=qb[:mb, :],
                                        in1=qc[:mb, :], op=ALU.add)
                term(qc, 6)
                nc.vector.tensor_tensor(out=qb[:mb, :], in0=qb[:mb, :],
                                        in1=qc[:mb, :], op=ALU.add)
                nc.vector.scalar_tensor_tensor(
                    out=qc[:mb, :], in0=qa[:mb, :],
                    scalar=c0_t[ci][:mb, 0:1], in1=qb[:mb, :],
                    op0=ALU.add, op1=ALU.add)
                nc.sync.dma_start(out=out_d[m0:m0 + mb, jb * BT:(jb + 1) * BT],
                                  in_=ot[:mb, :])
                nc.sync.dma_start(out=qnt_d[m0:m0 + mb, jb * BT:(jb + 1) * BT],
                                  in_=qc[:mb, :])
                m0 += mb

    nc.finalize()
    return nc


_CACHE = {}


def _get_program(f32r=True):
    key = ("nc", f32r)
    if key not in _CACHE:
        _CACHE[key] = build_program(f32r=f32r)
    return _CACHE[key]


def _prep_inputs(inputs):
    f32 = np.float32
    x = np.ascontiguousarray(np.asarray(inputs["x"], f32))

    shared = {}
    # The device computes -mish, so the layers consuming h (sl2_w0, sl3_w0,
    # fc_w) get negated weights; BN folding is self-consistent with this.
    NEG = {"sl2_w0", "sl3_w0", "fc_w"}
    for ln, shapes in SL_SHAPES.items():
        for i in range(len(shapes)):
            w = np.asarray(inputs[f"{ln}_w{i}"], f32)
            if f"{ln}_w{i}" in NEG:
                w = -w
            b = np.asarray(inputs[f"{ln}_b{i}"], f32)
            shared[f"{ln}_w{i}t"] = np.ascontiguousarray(w.T)
            shared[f"{ln}_b{i}"] = np.ascontiguousarray(b.reshape(-1, 1))
    fw = -np.asarray(inputs["fc_w"], f32)
    shared["fc_wt"] = np.ascontiguousarray(fw.T)
    shared["fc_b"] = np.ascontiguousarray(
        np.asarray(inputs["fc_b"], f32).reshape(-1, 1))
    shared["sl2_0_rs"] = np.ascontiguousarray(
        (-np.asarray(inputs["sl2_w0"], f32)).sum(axis=1).reshape(-1, 1))
    shared["sl3_0_rs"] = np.ascontiguousarray(
        (-np.asarray(inputs["sl3_w0"], f32)).sum(axis=1).reshape(-1, 1))
    shared["fc_rs"] = np.ascontiguousarray(fw.sum(axis=1).reshape(-1, 1))
    cb = np.asarray(inputs["codebook"], f32)[0, :, 0, :]
    cs = np.sort(cb, axis=1)
    shared["mids"] = np.ascontiguousarray((cs[:, :-1] + cs[:, 1:]) * 0.5)
    shared["c0"] = np.ascontiguousarray(cs[:, 0:1])
    shared["dts"] = np.ascontiguousarray(cs[:, 1:] - cs[:, :-1])

    in_maps = []
    for c in range(N_CORES):
        m = dict(shared)
        m["xt"] = np.ascontiguousarray(x[c * B_CORE:(c + 1) * B_CORE, :].T)
        in_maps.append(m)
    return in_maps


def run(inputs, trace=False, f32r=True):
    nc = _get_program(f32r=f32r)
    in_maps = _prep_inputs(inputs)
    res = run_bass_kernel_spmd(nc, in_maps, list(range(N_CORES)), trace=trace)
    outs = np.empty((FB, B_TOTAL), np.float32)
    qnts = np.empty((FB, B_TOTAL), np.float32)
    for c in range(N_CORES):
        outs[:, c * B_CORE:(c + 1) * B_CORE] = res.results[c]["out_t"]
        qnts[:, c * B_CORE:(c + 1) * B_CORE] = res.results[c]["qnt_t"]
    out = np.ascontiguousarray(outs.T)
    qnt = np.ascontiguousarray(qnts.T)
    return (qnt.copy(), qnt.reshape(B_TOTAL, FB, 1),
            out.reshape(B_TOTAL, FB, 1)), res


def kernel(**inputs):
    (strite, qnt, out), _ = run(inputs, trace=False)
    return strite, qnt, out


# revision 20
# speedup vs baseline: 1.5949x; 1.3560x over previous
"""Trainium2 Bass kernel for nn_MinEncoderOrder (vq_codebook).

Data-parallel over 8 NeuronCores: batch 32768 -> 4096/core. Weights and the
(512,8) codebook replicated. The three global BatchNorms are handled with
on-device AllReduce of (sum, sumsq) scalars; the BN affine is folded into the
next layer's PSUM->SBUF activation pass (per-partition scale/bias), so matmuls
never stall on stats.

Layout: activations feature-major [F, B_tile] (features on partitions, batch on
the free dim, 512-column batch tiles). Weights host-pre-transposed to [K, M]
and used as the stationary matmul operand.

mish(x) = x * (q-1)/(q+1) with q = (exp(x)+1)^2  [exact identity]:
2 ACT passes (exp, square) + 4 DVE passes, with BN sum/sumsq picked up for
free via scalar_tensor_tensor accum_out.

VQ: per-feature sorted codebook -> quant = c0 + sum_i delta_i * [out >= mid_i],
which reproduces the reference's argmin-with-max-on-ties semantics.
"""

import sys
import numpy as np

for _p in ("/opt/trn_rl_repo", "/root/.axon_site/_ro/trn_rl_repo"):
    if _p not in sys.path:
        sys.path.append(_p)

from contextlib import ExitStack

import concourse.bass as bass
import concourse.tile as tile
from concourse import bacc, mybir
from concourse.bass_utils import run_bass_kernel_spmd

F32 = mybir.dt.float32
AF = mybir.ActivationFunctionType
ALU = mybir.AluOpType

B_TOTAL = 32768
N_CORES = 8
B_CORE = B_TOTAL // N_CORES          # 4096
BT = 512                              # batch tile (one PSUM bank of fp32)
N_BT = B_CORE // BT                   # 8
EPS = 1e-5
FB = 512

SL_SHAPES = {
    "sl1": [(300, 24), (150, 150), (50, 50)],
    "sl2": [(600, 300), (300, 300), (100, 100)],
    "sl3": [(600, 600), (300, 300), (100, 100)],
}

# Output-row blocking per layer (PSUM partition chunks, aligned to the split
# boundary where that is free).
M_BLOCKS = {
    "sl1_0": [128, 22, 128, 22], "sl1_1": [100, 50], "sl1_2": [50],
    "sl2_0": [128, 128, 44, 128, 128, 44], "sl2_1": [128, 72, 100],
    "sl2_2": [100],
    "sl3_0": [128, 128, 44, 128, 128, 44], "sl3_1": [128, 72, 100],
    "sl3_2": [100],
    "fc": [128, 128, 128, 128],
}
# Contraction-dim chunking (must match the rhs tile segmentation; every
# segment must start at partition 0 of its tile — matmul moving-operand
# base-partition restriction).
K_SEGS = {
    "sl1_0": [24], "sl1_1": [128, 22], "sl1_2": [50],
    "sl2_0": [128, 128, 44], "sl2_1": [128, 128, 44], "sl2_2": [100],
    "sl3_0": [128, 128, 128, 128, 88], "sl3_1": [128, 128, 44], "sl3_2": [100],
    "fc": [128, 128, 128, 128, 88],
}
W_NAME = {"sl1_0": "sl1_w0t", "sl1_1": "sl1_w1t", "sl1_2": "sl1_w2t",
          "sl2_0": "sl2_w0t", "sl2_1": "sl2_w1t", "sl2_2": "sl2_w2t",
          "sl3_0": "sl3_w0t", "sl3_1": "sl3_w1t", "sl3_2": "sl3_w2t",
          "fc": "fc_wt"}


class Vec:
    """Feature-major vector stored as row-block tiles [p<=128, BT]."""

    def __init__(self, tiles, sizes):
        self.tiles = tiles
        self.sizes = sizes
        self.offsets = np.concatenate([[0], np.cumsum(sizes)]).astype(int)
        self.total = int(self.offsets[-1])

    def segments(self, lo, hi):
        for i, sz in enumerate(self.sizes):
            b0, b1 = self.offsets[i], self.offsets[i + 1]
            s0, s1 = max(lo, b0), min(hi, b1)
            if s0 < s1:
                yield self.tiles[i], int(s0 - b0), int(s1 - b0), int(s0)


def _pairs(src: Vec, src_lo, src_hi, dst: Vec, dst_lo):
    out = []
    for st, t0, t1, g0 in src.segments(src_lo, src_hi):
        cur_g, cur_t, n = g0, t0, t1 - t0
        while n > 0:
            d_global = dst_lo + (cur_g - src_lo)
            for dt_, u0, u1, _ in dst.segments(d_global, d_global + n):
                out.append((st, cur_t, cur_t + (u1 - u0), dt_, u0, u1))
                cur_t += u1 - u0
                cur_g += u1 - u0
                n -= u1 - u0
                break  # re-evaluate segments from the new position
    return out


def build_program(f32r=True):
    """f32r=True streams matmul operands in float32r (12-bit mantissa,
    1 column/cycle) instead of fp32 (4 cycles/column)."""
    mm_dt = mybir.dt.float32r if f32r else F32
    nc = bacc.Bacc(None, target_bir_lowering=False, num_devices=N_CORES)

    dram_in = {}

    def din(name, shape, dt=F32):
        dram_in[name] = nc.declare_dram_parameter(name, list(shape), dt,
                                                  isOutput=False)
        return dram_in[name]

    xt_d = din("xt", (24, B_CORE), mm_dt)
    for ln, shapes in SL_SHAPES.items():
        for i, (o, k) in enumerate(shapes):
            din(f"{ln}_w{i}t", (k, o), mm_dt)
            din(f"{ln}_b{i}", (o, 1))
    din("fc_wt", (600, FB), mm_dt)
    din("fc_b", (FB, 1))
    for n, rows in (("sl2_0", 600), ("sl3_0", 600), ("fc", FB)):
        din(f"{n}_rs", (rows, 1))
    mids_d = din("mids", (FB, 7))
    c0_d = din("c0", (FB, 1))
    dts_d = din("dts", (FB, 7))

    out_d = nc.declare_dram_parameter("out_t", [FB, B_CORE], F32, isOutput=True)
    qnt_d = nc.declare_dram_parameter("qnt_t", [FB, B_CORE], F32, isOutput=True)

    core_ids = list(range(N_CORES))
    BIAS_NAME = {"sl1_0": "sl1_b0", "sl1_1": "sl1_b1", "sl1_2": "sl1_b2",
                 "sl2_0": "sl2_b0", "sl2_1": "sl2_b1", "sl2_2": "sl2_b2",
                 "sl3_0": "sl3_b0", "sl3_1": "sl3_b1", "sl3_2": "sl3_b2",
                 "fc": "fc_b"}

    with tile.TileContext(nc) as tc, ExitStack() as ctx:
        wpool = ctx.enter_context(tc.tile_pool(name="w", bufs=1))
        hpool = ctx.enter_context(tc.tile_pool(name="h", bufs=44))
        rpool = ctx.enter_context(tc.tile_pool(name="r", bufs=10))
        spool = ctx.enter_context(tc.tile_pool(name="s", bufs=2))
        opool = ctx.enter_context(tc.tile_pool(name="o", bufs=2))
        stpool = ctx.enter_context(tc.tile_pool(name="st", bufs=1))
        dpool = ctx.enter_context(tc.tile_pool(name="d", bufs=1, space="DRAM"))
        psum = ctx.enter_context(tc.tile_pool(name="ps", bufs=6, space="PSUM"))
        pstat = ctx.enter_context(tc.tile_pool(name="pst", bufs=2, space="PSUM"))

        # ---- weights: one tile per K-chunk, rows matching K_SEGS ----
        wtiles = {}
        for name, segs in K_SEGS.items():
            d = dram_in[W_NAME[name]]
            tiles, r = [], 0
            for kk in segs:
                t = wpool.tile([kk, d.shape[1]], mm_dt, tag=f"w_{name}_{r}")
                nc.sync.dma_start(out=t, in_=d[r:r + kk, :])
                tiles.append(t)
                r += kk
            assert r == d.shape[0]
            wtiles[name] = tiles

        # ---- per-M-chunk bias tiles ----
        btiles = {}
        for name, mbs in M_BLOCKS.items():
            d = dram_in[BIAS_NAME[name]]
            tiles, m0 = [], 0
            for mb in mbs:
                t = wpool.tile([mb, 1], F32, tag=f"b_{name}_{m0}")
                nc.sync.dma_start(out=t, in_=d[m0:m0 + mb, :])
                tiles.append(t)
                m0 += mb
            btiles[name] = tiles

        # ---- per-M-chunk rowsum tiles (BN-folded layers) ----
        rstiles = {}
        for name in ("sl2_0", "sl3_0", "fc"):
            d = dram_in[f"{name}_rs"]
            tiles, m0 = [], 0
            for mb in M_BLOCKS[name]:
                t = wpool.tile([mb, 1], F32, tag=f"rs_{name}_{m0}")
                nc.sync.dma_start(out=t, in_=d[m0:m0 + mb, :])
                tiles.append(t)
                m0 += mb
            rstiles[name] = tiles

        # ---- VQ constants, 128-row chunks (fc M blocks are 128-aligned) ----
        def load_chunks(d, width, tag):
            tiles = []
            for i in range(FB // 128):
                t = wpool.tile([128, width], F32, tag=f"{tag}_{i}")
                nc.sync.dma_start(out=t, in_=d[i * 128:(i + 1) * 128, :])
                tiles.append(t)
            return tiles

        mids_t = load_chunks(mids_d, 7, "mids")
        c0_t = load_chunks(c0_d, 1, "c0")
        dts_t = load_chunks(dts_d, 7, "dts")

        ones_t = wpool.tile([128, 1], F32, tag="ones")
        nc.vector.memset(ones_t, 1.0)

        # ---- stats staging ----
        NSTAT = 352
        stats_t = stpool.tile([128, NSTAT], F32, tag="stats")
        nc.vector.memset(stats_t, 0.0)
        stat_col = [0]
        stat_ranges = {}

        def stat_cols(skey):
            c = stat_col[0]
            stat_col[0] += 1
            assert stat_col[0] <= NSTAT
            lo, hi = stat_ranges.get(skey, (c, c))
            stat_ranges[skey] = (min(lo, c), c + 1)
            return c

        s_bcast = [wpool.tile([128, 1], F32, tag=f"s_b{k}", name=f"s_b{k}") for k in range(3)]
        sm_bcast = [wpool.tile([128, 1], F32, tag=f"sm_b{k}", name=f"sm_b{k}") for k in range(3)]

        # ---------------- helpers ----------------
        def matmul_layer(name, rhs_vec, rhs_lo, rhs_hi):
            segs = list(rhs_vec.segments(rhs_lo, rhs_hi))
            assert [t1 - t0 for _, t0, t1, _ in segs] == K_SEGS[name], \
                f"{name}: rhs segs {[t1 - t0 for _, t0, t1, _ in segs]}"
            ps_tiles, m0 = [], 0
            for mb in M_BLOCKS[name]:
                ps = psum.tile([128, BT], F32, tag="mm")
                for si, ((rt, t0, t1, _), wt) in enumerate(zip(segs, wtiles[name])):
                    nc.tensor.matmul(
                        ps[:mb, :], wt[:, m0:m0 + mb], rt[t0:t1, 0:BT],
                        start=(si == 0), stop=(si == len(segs) - 1))
                ps_tiles.append((ps, mb))
                m0 += mb
            return ps_tiles

        def relu_drain(ps_tiles, name, bn=None):
            tiles, sizes, m0 = [], [], 0
            for ci, (ps, mb) in enumerate(ps_tiles):
                rt = rpool.tile([128, BT], mm_dt, tag="r", name="r")
                if bn is None:
                    nc.scalar.activation(rt[:mb, :], ps[:mb, :], AF.Relu,
                                         bias=btiles[name][ci][:, 0:1], scale=1.0)
                else:
                    k, beta = bn
                    nc.scalar.activation(rt[:mb, :], ps[:mb, :], AF.Relu,
                                         bias=beta[ci][:, 0:1],
                                         scale=s_bcast[k][:mb, 0:1])
                tiles.append(rt)
                sizes.append(mb)
                m0 += mb
            return Vec(tiles, sizes)

        def make_beta(name, k):
            """beta_chunk = bias - rowsum(W)*s*m per output feature."""
            betas = []
            for ci, mb in enumerate(M_BLOCKS[name]):
                bt_ = wpool.tile([mb, 1], F32, tag=f"beta_{name}_{ci}")
                nc.vector.scalar_tensor_tensor(
                    out=bt_[:, 0:1], in0=rstiles[name][ci][:, 0:1], scalar=-1.0,
                    in1=sm_bcast[k][:mb, 0:1], op0=ALU.mult, op1=ALU.mult)
                nc.vector.tensor_tensor(out=bt_[:, 0:1], in0=bt_[:, 0:1],
                                        in1=btiles[name][ci][:, 0:1], op=ALU.add)
                betas.append(bt_)
            return betas

        def mish_block(src, src_hi, dst, dst_lo, skey):
            """dst[dst_lo:dst_lo+src_hi) = -mish(src[0:src_hi)).

            -mish(x) = x*(1 - 2/(1+sigmoid(-x)^2)) for x>=0; the sign is
            compensated by host-side negation of the consumer weights.
            Compute ops only touch partition-0-based slices (hardware
            alignment rule); an SBUF->SBUF DMA repacks into the h layout."""
            for (st, t0, t1, g0) in src.segments(0, src_hi):
                assert t0 == 0
                p = t1 - t0
                xa = st[0:p, 0:BT]
                if f32r:
                    # non-matmul engines read the (already-rounded) bits as f32
                    xa = xa.bitcast(F32)
                u = spool.tile([128, BT], F32, tag="msA", name="msA")
                q = spool.tile([128, BT], F32, tag="msB", name="msB")
                a = spool.tile([128, BT], F32, tag="msC", name="msC")
                mo = spool.tile([128, BT], mm_dt, tag="msD", name="msD")
                nc.scalar.activation(u[:p, :], xa, AF.Sigmoid, scale=-1.0)
                nc.scalar.activation(q[:p, :], u[:p, :], AF.Square)
                nc.vector.tensor_scalar(out=a[:p, :], in0=q[:p, :], scalar1=1.0,
                                        scalar2=None, op0=ALU.add)
                nc.vector.reciprocal_approx_fast(out=a[:p, :], in_=a[:p, :])
                nc.vector.tensor_scalar(out=a[:p, :], in0=a[:p, :], scalar1=-2.0,
                                        scalar2=1.0, op0=ALU.mult, op1=ALU.add)
                c_sum = stat_cols(skey)
                nc.vector.scalar_tensor_tensor(
                    out=mo[:p, :], in0=a[:p, :], scalar=1.0, in1=xa,
                    op0=ALU.mult, op1=ALU.mult,
                    accum_out=stats_t[:p, c_sum:c_sum + 1])
                c_sq = stat_cols(skey)
                mo_r = mo[:p, :].bitcast(F32) if f32r else mo[:p, :]
                nc.vector.scalar_tensor_tensor(
                    out=u[:p, :], in0=mo_r, scalar=1.0, in1=mo_r,
                    op0=ALU.mult, op1=ALU.mult,
                    accum_out=stats_t[:p, c_sq:c_sq + 1])
                d0 = dst_lo + g0
                for dt_, u0, u1, hg in dst.segments(d0, d0 + p):
                    nc.gpsimd.dma_start(
                        out=dt_[u0:u1, 0:BT],
                        in_=mo[hg - d0:hg - d0 + (u1 - u0), 0:BT])

        def bn_reduce(k, skey, count):
            lo, hi = stat_ranges[skey]
            ncols = hi - lo
            nsum = pstat.tile([1, ncols], F32, tag="pst")
            nc.tensor.matmul(nsum[0:1, :], ones_t[:, 0:1], stats_t[:, lo:hi],
                             start=True, stop=True)
            srow = stpool.tile([1, ncols], F32, tag=f"srow{k}")
            nc.vector.tensor_copy(srow[0:1, :], nsum[0:1, :])
            tot = stpool.tile([1, 2], F32, tag=f"tot{k}")
            sview = srow[0:1, :].rearrange("p (n two) -> p n two", two=2)
            nc.vector.tensor_reduce(out=tot[0:1, 0:1], in_=sview[0:1, :, 0:1],
                                    axis=mybir.AxisListType.XY, op=ALU.add)
            nc.vector.tensor_reduce(out=tot[0:1, 1:2], in_=sview[0:1, :, 1:2],
                                    axis=mybir.AxisListType.XY, op=ALU.add)
            cc_in = dpool.tile([1, 2], F32, tag=f"ccin{k}")
            cc_out = dpool.tile([1, 2], F32, tag=f"ccout{k}")
            nc.sync.dma_start(out=cc_in[:, :], in_=tot[0:1, :])
            nc.gpsimd.collective_compute(
                "AllReduce", ALU.add, replica_groups=[core_ids],
                ins=[cc_in[:, :]], outs=[cc_out[:, :]])
            g = stpool.tile([1, 2], F32, tag=f"g{k}")
            nc.sync.dma_start(out=g[0:1, :], in_=cc_out[:, :])
            inv_n = 1.0 / float(count)
            mm_ = stpool.tile([1, 4], F32, tag=f"mm{k}")
            nc.vector.tensor_scalar(out=mm_[0:1, 0:2], in0=g[0:1, 0:2],
                                    scalar1=inv_n, scalar2=None, op0=ALU.mult)
            nc.vector.scalar_tensor_tensor(out=mm_[0:1, 2:3], in0=mm_[0:1, 0:1],
                                           scalar=-1.0, in1=mm_[0:1, 0:1],
                                           op0=ALU.mult, op1=ALU.mult)
            nc.vector.tensor_tensor(out=mm_[0:1, 2:3], in0=mm_[0:1, 2:3],
                                    in1=mm_[0:1, 1:2], op=ALU.add)
            nc.vector.tensor_scalar(out=mm_[0:1, 2:3], in0=mm_[0:1, 2:3],
                                    scalar1=EPS, scalar2=None, op0=ALU.add)
            # rsqrt(v): int bit-hack seed + 3 Newton iterations (all DVE)
            iv = mm_[0:1, 2:3]
            y = stpool.tile([1, 1], F32, tag=f"y{k}")
            nc.vector.tensor_scalar(out=y[0:1, 0:1].bitcast(mybir.dt.int32),
                                    in0=iv.bitcast(mybir.dt.int32), scalar1=1,
                                    scalar2=None, op0=ALU.logical_shift_right)
            nc.vector.tensor_scalar(out=y[0:1, 0:1].bitcast(mybir.dt.int32),
                                    in0=y[0:1, 0:1].bitcast(mybir.dt.int32),
                                    scalar1=-1, scalar2=None,
                                    op0=ALU.bitwise_xor)
            nc.vector.tensor_scalar(out=y[0:1, 0:1].bitcast(mybir.dt.int32),
                                    in0=y[0:1, 0:1].bitcast(mybir.dt.int32),
                                    scalar1=0x5F3759E0, scalar2=None,
                                    op0=ALU.add)
            t1_ = stpool.tile([1, 1], F32, tag=f"t1{k}")
            for _ in range(3):
                nc.vector.tensor_tensor(out=t1_[0:1, 0:1], in0=y[0:1, 0:1],
                                        in1=y[0:1, 0:1], op=ALU.mult)
                nc.vector.scalar_tensor_tensor(out=t1_[0:1, 0:1], in0=iv,
                                               scalar=-0.5, in1=t1_[0:1, 0:1],
                                               op0=ALU.mult, op1=ALU.mult)
                nc.vector.tensor_scalar(out=t1_[0:1, 0:1], in0=t1_[0:1, 0:1],
                                        scalar1=1.5, scalar2=None, op0=ALU.add)
                nc.vector.tensor_tensor(out=y[0:1, 0:1], in0=y[0:1, 0:1],
                                        in1=t1_[0:1, 0:1], op=ALU.mult)
            sm_ = stpool.tile([1, 2], F32, tag=f"smv{k}")
            nc.vector.tensor_copy(sm_[0:1, 0:1], y[0:1, 0:1])
            nc.vector.tensor_tensor(out=sm_[0:1, 1:2], in0=y[0:1, 0:1],
                                    in1=mm_[0:1, 0:1], op=ALU.mult)
            sc = dpool.tile([1, 2], F32, tag=f"sc{k}")
            nc.sync.dma_start(out=sc[:, :], in_=sm_[0:1, :])
            nc.sync.dma_start(out=s_bcast[k][:, 0:1],
                              in_=sc[0:1, 0:1].partition_broadcast(128))
            nc.sync.dma_start(out=sm_bcast[k][:, 0:1],
                              in_=sc[0:1, 1:2].partition_broadcast(128))

        # ---------------- the network ----------------
        def sl_phase(ln, in_vecs, h_sizes, bn_in, skey):
            """One split-layer block for all batch tiles -> dict of h Vecs."""
            h_all = {}
            o0, o1, o2 = {"sl1": (150, 100, 50), "sl2": (300, 200, 100),
                          "sl3": (300, 200, 100)}[ln]
            n0 = {"sl1": 300, "sl2": 600, "sl3": 600}[ln]
            n1 = {"sl1": 150, "sl2": 300, "sl3": 300}[ln]
            n2 = {"sl1": 50, "sl2": 100, "sl3": 100}[ln]
            for jb in range(N_BT):
                h = Vec([hpool.tile([128, BT], mm_dt, tag="h", name="h") for _ in h_sizes],
                        h_sizes)
                h_all[jb] = h
                ps = matmul_layer(f"{ln}_0", in_vecs[jb], 0, in_vecs[jb].total)
                r0 = relu_drain(ps, f"{ln}_0", bn=bn_in)
                ps = matmul_layer(f"{ln}_1", r0, o0, n0)
                r1 = relu_drain(ps, f"{ln}_1")
                ps = matmul_layer(f"{ln}_2", r1, o1, n1)
                r2 = relu_drain(ps, f"{ln}_2")
                mish_block(r0, o0, h, 0, skey)
                mish_block(r1, o1, h, o0, skey)
                mish_block(r2, n2, h, o0 + o1, skey)
            return h_all

        # phase 1
        xvecs = {}
        for jb in range(N_BT):
            xt_t = rpool.tile([24, BT], mm_dt, tag="xt", bufs=3)
            nc.sync.dma_start(out=xt_t, in_=xt_d[:, jb * BT:(jb + 1) * BT])
            xvecs[jb] = Vec([xt_t], [24])
        h1 = sl_phase("sl1", xvecs, [128, 128, 44], None, "bn1")
        bn_reduce(0, "bn1", B_TOTAL * 300)
        beta_sl2 = make_beta("sl2_0", 0)

        h2 = sl_phase("sl2", h1, [128, 128, 128, 128, 88], (0, beta_sl2), "bn2")
        bn_reduce(1, "bn2", B_TOTAL * 600)
        beta_sl3 = make_beta("sl3_0", 1)

        h3 = sl_phase("sl3", h2, [128, 128, 128, 128, 88], (1, beta_sl3), "bn3")
        bn_reduce(2, "bn3", B_TOTAL * 600)
        beta_fc = make_beta("fc", 2)

        # phase 4: fc + sigmoid + VQ
        for jb in range(N_BT):
            ps = matmul_layer("fc", h3[jb], 0, 600)
            m0 = 0
            for ci, (pst_, mb) in enumerate(ps):
                ot = opool.tile([128, BT], F32, tag="out")
                nc.scalar.activation(ot[:mb, :], pst_[:mb, :], AF.Sigmoid,
                                     bias=beta_fc[ci][:, 0:1],
                                     scale=s_bcast[2][:mb, 0:1])
                # quant = c0 + sum_i dts_i*[out >= mids_i]; fused compare+scale
                # at 2x TS rate, tree-sum split between DVE and GpSimd.
                qa = opool.tile([128, BT], F32, tag="qA", name="qA")
                qb = opool.tile([128, BT], F32, tag="qB", name="qB")
                qc = opool.tile([128, BT], F32, tag="qC", name="qC")

                def term(dst_t, t):
                    nc.vector.tensor_scalar(
                        out=dst_t[:mb, :], in0=ot[:mb, :],
                        scalar1=mids_t[ci][:mb, t:t + 1],
                        scalar2=dts_t[ci][:mb, t:t + 1],
                        op0=ALU.is_ge, op1=ALU.mult)

                term(qa, 0)
                term(qb, 1)
                nc.vector.tensor_tensor(out=qa[:mb, :], in0=qa[:mb, :],
                                        in1=qb[:mb, :], op=ALU.add)
                term(qb, 2)
                term(qc, 3)
                nc.vector.tensor_tensor(out=qb[:mb, :], in0=qb[:mb, :],
                                        in1=qc[:mb, :], op=ALU.add)
                nc.vector.tensor_tensor(out=qa[:mb, :], in0=qa[:mb, :],
                                        in1=qb[:mb, :], op=ALU.add)
                term(qb, 4)
                term(qc, 5)
                nc.vector.tensor_tensor(out=qb[:mb, :], in0=qb[:mb, :],
                                        in1=qc[:mb, :], op=ALU.add)
                term(qc, 6)
                nc.vector.tensor_tensor(out=qb[:mb, :], in0=qb[:mb, :],
                                        in1=qc[:mb, :], op=ALU.add)
                nc.vector.scalar_tensor_tensor(
                    out=qc[:mb, :], in0=qa[:mb, :],
                    scalar=c0_t[ci][:mb, 0:1], in1=qb[:mb, :],
                    op0=ALU.add, op1=ALU.add)
                nc.sync.dma_start(out=out_d[m0:m0 + mb, jb * BT:(jb + 1) * BT],
                                  in_=ot[:mb, :])
                nc.sync.dma_start(out=qnt_d[m0:m0 + mb, jb * BT:(jb + 1) * BT],
                                  in_=qc[:mb, :])
                m0 += mb

    nc.finalize()
    return nc


_CACHE = {}


def _get_program(f32r=True):
    key = ("nc", f32r)
    if key not in _CACHE:
        _CACHE[key] = build_program(f32r=f32r)
    return _CACHE[key]


def _prep_inputs(inputs):
    f32 = np.float32
    x = np.ascontiguousarray(np.asarray(inputs["x"], f32))

    shared = {}
    # The device computes -mish, so the layers consuming h (sl2_w0, sl3_w0,
    # fc_w) get negated weights; BN folding is self-consistent with this.
    NEG = {"sl2_w0", "sl3_w0", "fc_w"}
    for ln, shapes in SL_SHAPES.items():
        for i in range(len(shapes)):
            w = np.asarray(inputs[f"{ln}_w{i}"], f32)
            if f"{ln}_w{i}" in NEG:
                w = -w
            b = np.asarray(inputs[f"{ln}_b{i}"], f32)
            shared[f"{ln}_w{i}t"] = np.ascontiguousarray(w.T)
            shared[f"{ln}_b{i}"] = np.ascontiguousarray(b.reshape(-1, 1))
    fw = -np.asarray(inputs["fc_w"], f32)
    shared["fc_wt"] = np.ascontiguousarray(fw.T)
    shared["fc_b"] = np.ascontiguousarray(
        np.asarray(inputs["fc_b"], f32).reshape(-1, 1))
    shared["sl2_0_rs"] = np.ascontiguousarray(
        (-np.asarray(inputs["sl2_w0"], f32)).sum(axis=1).reshape(-1, 1))
    shared["sl3_0_rs"] = np.ascontiguousarray(
        (-np.asarray(inputs["sl3_w0"], f32)).sum(axis=1).reshape(-1, 1))
    shared["fc_rs"] = np.ascontiguousarray(fw.sum(axis=1).reshape(-1, 1))
    cb = np.asarray(inputs["codebook"], f32)[0, :, 0, :]
    cs = np.sort(cb, axis=1)
    shared["mids"] = np.ascontiguousarray((cs[:, :-1] + cs[:, 1:]) * 0.5)
    shared["c0"] = np.ascontiguousarray(cs[:, 0:1])
    shared["dts"] = np.ascontiguousarray(cs[:, 1:] - cs[:, :-1])

    in_maps = []
    for c in range(N_CORES):
        m = dict(shared)
        m["xt"] = np.ascontiguousarray(x[c * B_CORE:(c + 1) * B_CORE, :].T)
        in_maps.append(m)
    return in_maps


def run(inputs, trace=False, f32r=True):
    nc = _get_program(f32r=f32r)
    in_maps = _prep_inputs(inputs)
    res = run_bass_kernel_spmd(nc, in_maps, list(range(N_CORES)), trace=trace)
    outs = np.empty((FB, B_TOTAL), np.float32)
    qnts = np.empty((FB, B_TOTAL), np.float32)
    for c in range(N_CORES):
        outs[:, c * B_CORE:(c + 1) * B_CORE] = res.results[c]["out_t"]
        qnts[:, c * B_CORE:(c + 1) * B_CORE] = res.results[c]["qnt_t"]
    out = np.ascontiguousarray(outs.T)
    qnt = np.ascontiguousarray(qnts.T)
    return (qnt.copy(), qnt.reshape(B_TOTAL, FB, 1),
            out.reshape(B_TOTAL, FB, 1)), res


def kernel(**inputs):
    (strite, qnt, out), _ = run(inputs, trace=False)
    return strite, qnt, out
